# revision 10
# baseline (speedup 1.0000x reference)
"""Trainium2 Bass kernel for nn_ComplexAttention (B=8, C=512, H=W=32, HEADS=8).

Strategy
--------
Data-parallel over batch: one batch element per NeuronCore (8 cores), no
collectives.  Host-side algebraic fusion shrinks the per-core work:

  reference:  Q = R_q Wq Z,  K = R_k Wk Z,  V = R_v Wv Z   (complex, [C,T])
              S = Re(Q^H K)/sqrt(dh),  causal softmax -> A
              out = R_o Wo (V A^T)

  fused:      M = Wq^T diag(e^{i(phi_k-phi_q)}) Wk / sqrt(dh)   (host, f64)
              N = diag(e^{i phi_o}) Wo diag(e^{i phi_v}) Wv     (host, f64)
              Y = M Z            (channel-major [C,T])
              S = Re(Z^H Y)      = Zre^T Yre + Zim^T Yim
              A = softmax(causal(S))        (no max-subtraction: |S| < ~30)
              U = N Z            (token-major [T,C])
              out = U^T A^T      (channel-major [C,T], = re/im pair)

Whole datapath is bf16 (PSUM accumulation f32): 1 cyc/row PE rate, half
the DMA/DVE bytes, exact causal clamping (no N>=256 f32r restriction).
Outputs are written bf16 and upcast on host (rel err ~8e-3 vs 2e-2 gate).

Schedule notes (from HW traces):
 - NEFF bootstrap is ~7us; DMA_DIRECT2D issue is ~0.6us each, so inputs
   are consolidated into 13 large DMAs (zre/zim as [128,1024] c-tiles,
   mtre as 1+3 row-tiles, ntre as one [128,4,512]).  The two tiles the
   first matmul needs (mtre0 / zre c0) go first on separate queues.
 - phase order: Yre, Ure, Yim, Uim (ntre is loaded before zim), then
   scores tiles 7,6,5,4 / 3 / out(1,h0) / 2 / out(1,h1) / 1,0 / out(0);
   the interleaved scores tiles hide softmax latency ahead of each out
   chunk, and out(1) drains its DMA under the scores 0..3 work.
 - softmax exp reads scores straight out of PSUM (no copy), per-chunk
   partial row-sums are added on DVE afterwards.
"""

import math

import numpy as np

import concourse.mybir as mybir
import concourse.tile as tile
from concourse import bacc
from concourse.bass_utils import run_bass_kernel_spmd

B, C, HH, WW = 8, 512, 32, 32
T = HH * WW          # 1024 tokens
DH = C // 8          # head dim (scale only)
P = 128
CT = C // P          # 4 channel tiles
TT = T // P          # 8 token tiles
NEG = -1.0e30

f32 = mybir.dt.float32
bf16 = mybir.dt.bfloat16


def _mm(nc, out, lhsT, rhs, start, stop):
    nc.tensor.matmul(out, lhsT, rhs, start=start, stop=stop)


_CACHE: dict = {}


def _get_program(has_imag: bool):
    key = has_imag
    if key not in _CACHE:
        _CACHE[key] = _build_program(has_imag)
    return _CACHE[key]


def _build_program(has_imag: bool):
    nc = bacc.Bacc("TRN2", target_bir_lowering=False, debug=False)

    zre_d = nc.dram_tensor("zre", [C, T], bf16, kind="ExternalInput").ap()
    zim_d = nc.dram_tensor("zim", [C, T], bf16, kind="ExternalInput").ap()
    mtre_d = nc.dram_tensor("mtre", [C, C], bf16, kind="ExternalInput").ap()
    ntre_d = nc.dram_tensor("ntre", [C, C], bf16, kind="ExternalInput").ap()
    if has_imag:
        mtim_d = nc.dram_tensor("mtim", [C, C], bf16, kind="ExternalInput").ap()
        mtimn_d = nc.dram_tensor("mtimn", [C, C], bf16, kind="ExternalInput").ap()
        ntim_d = nc.dram_tensor("ntim", [C, C], bf16, kind="ExternalInput").ap()
        ntimn_d = nc.dram_tensor("ntimn", [C, C], bf16, kind="ExternalInput").ap()
    ident_d = nc.dram_tensor("ident", [P, P], bf16, kind="ExternalInput").ap()
    tri_d = nc.dram_tensor("tri", [P, P], f32, kind="ExternalInput").ap()
    outre_d = nc.dram_tensor("outre", [C, T], bf16, kind="ExternalOutput").ap()
    outim_d = nc.dram_tensor("outim", [C, T], bf16, kind="ExternalOutput").ap()

    with tile.TileContext(nc) as tc:
        with (
            tc.tile_pool(name="const", bufs=1) as cp,
            tc.tile_pool(name="work", bufs=4) as wp,
            tc.tile_pool(name="small", bufs=12) as sp,
            tc.tile_pool(name="psmm", bufs=6, space="PSUM") as pmm,
            tc.tile_pool(name="pstr", bufs=2, space="PSUM") as ptr,
        ):
            # -- input tiles -----------------------------------------------
            zre_c = [cp.tile([P, T], bf16, tag=f"zre{c}", name=f"zre{c}")
                     for c in range(CT)]
            zim_c = [cp.tile([P, T], bf16, tag=f"zim{c}", name=f"zim{c}")
                     for c in range(CT)]
            mtre0 = cp.tile([P, C], bf16, tag="mtre0", name="mtre0")
            mtre123 = cp.tile([P, 3, C], bf16, tag="mtre123", name="mtre123")
            ntre_sb = cp.tile([P, CT, C], bf16, tag="ntre", name="ntre")
            ident = cp.tile([P, P], bf16, tag="ident", name="ident")
            tri = cp.tile([P, P], f32, tag="tri", name="tri")

            mt_aps = [mtre0] + [mtre123[:, i, :] for i in range(3)]
            nt_aps = [ntre_sb[:, c, :] for c in range(CT)]

            mview = mtre_d.rearrange("(m p) t -> p m t", p=P)

            # critical first loads split across three queues in parallel:
            # the first Y matmuls need only mtre0 + zre c0 half 0.
            nc.gpsimd.dma_start(out=zre_c[0][:, 0:512], in_=zre_d[0:P, 0:512])
            nc.scalar.dma_start(out=mtre0, in_=mtre_d[0:P, :])
            nc.gpsimd.dma_start(out=zre_c[0][:, 512:T],
                                in_=zre_d[0:P, 512:T])
            nc.scalar.dma_start(out=mtre123, in_=mview[:, 1:4, :])
            for c in range(1, CT):
                nc.sync.dma_start(out=zre_c[c],
                                  in_=zre_d[c * P:(c + 1) * P, :])
            nc.gpsimd.dma_start(out=ident, in_=ident_d)
            nc.gpsimd.dma_start(out=tri, in_=tri_d)
            nview = ntre_d.rearrange("(m p) t -> p m t", p=P)
            nc.scalar.dma_start(out=ntre_sb, in_=nview)
            for c in range(CT):
                nc.sync.dma_start(out=zim_c[c],
                                  in_=zim_d[c * P:(c + 1) * P, :])
            if has_imag:
                def load4(dram, tag):
                    t4 = cp.tile([P, CT, C], bf16, tag=tag, name=tag)
                    nc.sync.dma_start(
                        out=t4, in_=dram.rearrange("(m p) t -> p m t", p=P))
                    return [t4[:, c, :] for c in range(CT)]
                mtim = load4(mtim_d, "mtim")
                mtimn = load4(mtimn_d, "mtimn")
                ntim = load4(ntim_d, "ntim")
                ntimn = load4(ntimn_d, "ntimn")

            # persistent result tiles (y split by column half: precise deps)
            yre = [[cp.tile([P, 512], bf16, tag=f"yre{c}_{n}",
                            name=f"yre{c}_{n}") for n in range(2)]
                   for c in range(CT)]
            yim = [[cp.tile([P, 512], bf16, tag=f"yim{c}_{n}",
                            name=f"yim{c}_{n}") for n in range(2)]
                   for c in range(CT)]
            ure = [cp.tile([P, C], bf16, tag=f"ure{j}", name=f"ure{j}")
                   for j in range(TT)]
            uim = [cp.tile([P, C], bf16, tag=f"uim{j}", name=f"uim{j}")
                   for j in range(TT)]

            def emit_y(dst, terms):
                """dst[m][n] = sum_terms w^T z ([C,T] channel-major)."""
                nterm = len(terms)
                for n in range(2):
                    pss = [pmm.tile([P, 512], f32, tag="mm", name="psmm")
                           for _ in range(CT)]
                    for t_i, (w, z) in enumerate(terms):
                        for c in range(CT):
                            for m in range(CT):
                                _mm(nc, pss[m], w[c][:, m * P:(m + 1) * P],
                                    z[c][:, n * 512:(n + 1) * 512],
                                    start=(t_i == 0 and c == 0),
                                    stop=(t_i == nterm - 1 and c == CT - 1))
                    for m in range(CT):
                        nc.vector.tensor_copy(out=dst[m][n], in_=pss[m])

            def emit_u(dst, terms):
                """dst[j] = sum_terms z^T w ([T,C] token-major)."""
                for j in range(TT):
                    jsl = slice(j * P, (j + 1) * P)
                    ps = pmm.tile([P, 512], f32, tag="mm", name="psmm")
                    nacc = len(terms) * CT
                    k = 0
                    for z, w in terms:
                        for c in range(CT):
                            _mm(nc, ps, z[c][:, jsl], w[c],
                                start=(k == 0), stop=(k == nacc - 1))
                            k += 1
                    nc.vector.tensor_copy(out=dst[j], in_=ps)

            # -- P^T blocks: one tile per t-chunk, j-blocks side by side ----
            pt0 = cp.tile([P, 4, 512], bf16, tag="pt0", name="pt0")
            pt1 = cp.tile([P, 8, 512], bf16, tag="pt1", name="pt1")
            ptn = (pt0, pt1)

            def emit_out_chunk(n, half, c0=0, c1=512):
                """out[:, n*512+c0 : n*512+c1] = U^T @ P^T for re/im."""
                tsl = slice(n * 512 + c0, n * 512 + c1)
                u, dram = ((ure, outre_d), (uim, outim_d))[half]
                dview = dram.rearrange("(m p) t -> p m t", p=P)
                js = [j for j in range(4 * n + 4) if j * P - n * 512 < c1]
                for mh in range(2):
                    o = wp.tile([P, 2, c1 - c0], bf16, tag=f"osb{c1 - c0}",
                                name="osb")
                    for mi in range(2):
                        m = 2 * mh + mi
                        msl = slice(m * P, (m + 1) * P)
                        ps = pmm.tile([P, 512], f32, tag="mm", name="psmm")
                        for j in js:
                            # pt j-block is zero left of column j*P-n*512
                            lo = max(c0, j * P - n * 512)
                            _mm(nc, ps[:, lo:c1],
                                u[j][:, msl], ptn[n][:, j, lo:c1],
                                start=(j == js[0]), stop=(j == js[-1]))
                        nc.vector.tensor_copy(out=o[:, mi, :],
                                              in_=ps[:, c0:c1])
                    nc.sync.dma_start(out=dview[:, 2 * mh:2 * mh + 2, tsl],
                                      in_=o)

            # -- scores / softmax / transposes per t-tile -------------------
            def emit_scores_tile(i):
                ui = (i + 1) * P
                isl = slice(i * P, (i + 1) * P)
                s_sb = wp.tile([P, T], bf16, tag="s", name="s_sb")
                nchunks = (ui + 511) // 512
                lparts = []
                for q in range(nchunks):
                    w = min(512, ui - q * 512)
                    ps = pmm.tile([P, 512], f32, tag="mm", name="psmm")
                    k = 0
                    for z, y in ((zre_c, yre), (zim_c, yim)):
                        for c in range(CT):
                            _mm(nc, ps[:, :w], z[c][:, isl], y[c][q][:, :w],
                                start=(k == 0), stop=(k == 2 * CT - 1))
                            k += 1
                    if q == nchunks - 1:
                        # causal frontier: add the tri mask in-place (PSUM)
                        nc.vector.tensor_add(out=ps[:, w - P: w],
                                             in0=ps[:, w - P: w], in1=tri)
                    lp = sp.tile([P, 1], f32, tag="lp", name="lp")
                    nc.scalar.activation(
                        out=s_sb[:, q * 512: q * 512 + w],
                        in_=ps[:, :w],
                        func=mybir.ActivationFunctionType.Exp,
                        accum_out=lp,
                    )
                    lparts.append(lp)

                lsum = lparts[0]
                for extra in lparts[1:]:
                    acc = sp.tile([P, 1], f32, tag="lacc", name="lacc")
                    nc.vector.tensor_add(out=acc, in0=lsum, in1=extra)
                    lsum = acc
                rl = sp.tile([P, 1], f32, tag="rl", name="rl")
                nc.vector.reciprocal(out=rl, in_=lsum)

                # fold 1/l into the transpose: block^T @ diag(1/l) on the PE
                dg = sp.tile([P, P], bf16, tag="dg", name="dg")
                nc.vector.tensor_scalar_mul(dg, ident, rl)
                n = i // 4
                lc = i * P - n * 512
                for g in range(0, i + 1, 4):
                    cnt = min(4, i + 1 - g)
                    psq = ptr.tile([P, 4, P], f32, tag="tr", name="pstile")
                    for k in range(cnt):
                        j = g + k
                        _mm(nc, psq[:, k, :], s_sb[:, j * P:(j + 1) * P], dg,
                            start=True, stop=True)
                    nc.vector.tensor_copy(
                        out=ptn[n][:, g:g + cnt, lc:lc + P],
                        in_=psq[:, :cnt, :],
                    )

            # -- phase order ------------------------------------------------
            if not has_imag:
                emit_y(yre, [(mt_aps, zre_c)])
                emit_u(ure, [(zre_c, nt_aps)])
                emit_y(yim, [(mt_aps, zim_c)])
                emit_u(uim, [(zim_c, nt_aps)])
            else:
                emit_y(yre, [(mt_aps, zre_c), (mtimn, zim_c)])
                emit_u(ure, [(zre_c, nt_aps), (zim_c, ntimn)])
                emit_y(yim, [(mt_aps, zim_c), (mtim, zre_c)])
                emit_u(uim, [(zim_c, nt_aps), (zre_c, ntim)])

            for i in (3, 2, 1, 0, 4, 5):
                emit_scores_tile(i)
            emit_out_chunk(0, half=0)
            emit_scores_tile(6)
            emit_out_chunk(0, half=1)
            emit_scores_tile(7)
            # out chunk 1 split by column strip: cols 0:384 need only
            # scores tiles 4..6, so they run while tile 7's softmax drains
            emit_out_chunk(1, half=0, c0=0, c1=384)
            emit_out_chunk(1, half=1, c0=0, c1=384)
            emit_out_chunk(1, half=0, c0=384, c1=512)
            emit_out_chunk(1, half=1, c0=384, c1=512)

    nc.compile()
    return nc


def _prep_weights(Wq, phi_q, Wk, phi_k, Wv, phi_v, Wo, phi_o):
    Wq, Wk, Wv, Wo = (np.asarray(w, np.float64) for w in (Wq, Wk, Wv, Wo))
    pq, pk, pv, po = (np.asarray(p, np.float64)
                      for p in (phi_q, phi_k, phi_v, phi_o))
    M = (Wq.T @ (np.exp(1j * (pk - pq))[:, None] * Wk)) / math.sqrt(DH)
    N = (np.exp(1j * po)[:, None] * Wo) @ (np.exp(1j * pv)[:, None] * Wv)
    has_imag = not (np.allclose(M.imag, 0.0) and np.allclose(N.imag, 0.0))
    return M, N, has_imag


def kernel(z_re, z_im, Wq, phi_q, Wk, phi_k, Wv, phi_v, Wo, phi_o):
    import ml_dtypes
    snp = ml_dtypes.bfloat16
    z_re = np.ascontiguousarray(np.asarray(z_re, np.float32).astype(snp))
    z_im = np.ascontiguousarray(np.asarray(z_im, np.float32).astype(snp))
    M, N, has_imag = _prep_weights(Wq, phi_q, Wk, phi_k, Wv, phi_v, Wo, phi_o)

    mtre = np.ascontiguousarray(M.real.T.astype(snp))
    ntre = np.ascontiguousarray(N.real.T.astype(snp))
    consts = {"mtre": mtre, "ntre": ntre}
    if has_imag:
        mtim = np.ascontiguousarray(M.imag.T.astype(snp))
        ntim = np.ascontiguousarray(N.imag.T.astype(snp))
        consts.update(mtim=mtim, mtimn=-mtim, ntim=ntim, ntimn=-ntim)

    consts["ident"] = np.eye(P, dtype=snp)
    consts["tri"] = np.triu(np.full((P, P), NEG, np.float32), 1)
    nc = _get_program(has_imag)
    in_maps = [
        dict(consts, zre=z_re[b].reshape(C, T), zim=z_im[b].reshape(C, T))
        for b in range(B)
    ]
    res = run_bass_kernel_spmd(nc, in_maps, list(range(B)))
    out_re = np.stack([res.results[b]["outre"].astype(np.float32)
                       .reshape(C, HH, WW) for b in range(B)])
    out_im = np.stack([res.results[b]["outim"].astype(np.float32)
                       .reshape(C, HH, WW) for b in range(B)])
    return out_re, out_im


# revision 11
# speedup vs baseline: 1.0191x; 1.0191x over previous
"""Trainium2 Bass kernel for nn_ComplexAttention (B=8, C=512, H=W=32, HEADS=8).

Strategy
--------
Data-parallel over batch: one batch element per NeuronCore (8 cores), no
collectives.  Host-side algebraic fusion shrinks the per-core work:

  reference:  Q = R_q Wq Z,  K = R_k Wk Z,  V = R_v Wv Z   (complex, [C,T])
              S = Re(Q^H K)/sqrt(dh),  causal softmax -> A
              out = R_o Wo (V A^T)

  fused:      M = Wq^T diag(e^{i(phi_k-phi_q)}) Wk / sqrt(dh)   (host, f64)
              N = diag(e^{i phi_o}) Wo diag(e^{i phi_v}) Wv     (host, f64)
              Y = M Z            (channel-major [C,T])
              S = Re(Z^H Y)      = Zre^T Yre + Zim^T Yim
              A = softmax(causal(S))        (no max-subtraction: |S| < ~30)
              U = N Z            (token-major [T,C])
              out = U^T A^T      (channel-major [C,T], = re/im pair)

Whole datapath is bf16 (PSUM accumulation f32): 1 cyc/row PE rate, half
the DMA/DVE bytes, exact causal clamping (no N>=256 f32r restriction).
Outputs are written bf16 and upcast on host (rel err ~8e-3 vs 2e-2 gate).

Schedule notes (from HW traces):
 - NEFF bootstrap is ~7us; DMA_DIRECT2D issue is ~0.6us each, so inputs
   are consolidated into 13 large DMAs (zre/zim as [128,1024] c-tiles,
   mtre as 1+3 row-tiles, ntre as one [128,4,512]).  The two tiles the
   first matmul needs (mtre0 / zre c0) go first on separate queues.
 - phase order: Yre, Ure, Yim, Uim (ntre is loaded before zim), then
   scores tiles 7,6,5,4 / 3 / out(1,h0) / 2 / out(1,h1) / 1,0 / out(0);
   the interleaved scores tiles hide softmax latency ahead of each out
   chunk, and out(1) drains its DMA under the scores 0..3 work.
 - softmax exp reads scores straight out of PSUM (no copy), per-chunk
   partial row-sums are added on DVE afterwards.
"""

import math

import numpy as np

import concourse.mybir as mybir
import concourse.tile as tile
from concourse import bacc
from concourse.bass_utils import run_bass_kernel_spmd

B, C, HH, WW = 8, 512, 32, 32
T = HH * WW          # 1024 tokens
DH = C // 8          # head dim (scale only)
P = 128
CT = C // P          # 4 channel tiles
TT = T // P          # 8 token tiles
NEG = -1.0e30

f32 = mybir.dt.float32
bf16 = mybir.dt.bfloat16


def _mm(nc, out, lhsT, rhs, start, stop):
    nc.tensor.matmul(out, lhsT, rhs, start=start, stop=stop)


_CACHE: dict = {}


def _get_program(has_imag: bool):
    key = has_imag
    if key not in _CACHE:
        _CACHE[key] = _build_program(has_imag)
    return _CACHE[key]


def _build_program(has_imag: bool):
    nc = bacc.Bacc("TRN2", target_bir_lowering=False, debug=False)

    zre_d = nc.dram_tensor("zre", [C, T], bf16, kind="ExternalInput").ap()
    zim_d = nc.dram_tensor("zim", [C, T], bf16, kind="ExternalInput").ap()
    mtre_d = nc.dram_tensor("mtre", [C, C], bf16, kind="ExternalInput").ap()
    ntre_d = nc.dram_tensor("ntre", [C, C], bf16, kind="ExternalInput").ap()
    if has_imag:
        mtim_d = nc.dram_tensor("mtim", [C, C], bf16, kind="ExternalInput").ap()
        mtimn_d = nc.dram_tensor("mtimn", [C, C], bf16, kind="ExternalInput").ap()
        ntim_d = nc.dram_tensor("ntim", [C, C], bf16, kind="ExternalInput").ap()
        ntimn_d = nc.dram_tensor("ntimn", [C, C], bf16, kind="ExternalInput").ap()
    ident_d = nc.dram_tensor("ident", [P, P], bf16, kind="ExternalInput").ap()
    tri_d = nc.dram_tensor("tri", [P, P], f32, kind="ExternalInput").ap()
    outre_d = nc.dram_tensor("outre", [C, T], bf16, kind="ExternalOutput").ap()
    outim_d = nc.dram_tensor("outim", [C, T], bf16, kind="ExternalOutput").ap()

    with tile.TileContext(nc) as tc:
        with (
            tc.tile_pool(name="const", bufs=1) as cp,
            tc.tile_pool(name="work", bufs=4) as wp,
            tc.tile_pool(name="small", bufs=12) as sp,
            tc.tile_pool(name="psmm", bufs=6, space="PSUM") as pmm,
            tc.tile_pool(name="pstr", bufs=2, space="PSUM") as ptr,
        ):
            # -- input tiles -----------------------------------------------
            zre_c = [cp.tile([P, T], bf16, tag=f"zre{c}", name=f"zre{c}")
                     for c in range(CT)]
            zim_c = [cp.tile([P, T], bf16, tag=f"zim{c}", name=f"zim{c}")
                     for c in range(CT)]
            mtre0 = cp.tile([P, C], bf16, tag="mtre0", name="mtre0")
            mtre123 = cp.tile([P, 3, C], bf16, tag="mtre123", name="mtre123")
            ntre_sb = cp.tile([P, CT, C], bf16, tag="ntre", name="ntre")
            ident = cp.tile([P, P], bf16, tag="ident", name="ident")
            tri = cp.tile([P, P], f32, tag="tri", name="tri")

            mt_aps = [mtre0] + [mtre123[:, i, :] for i in range(3)]
            nt_aps = [ntre_sb[:, c, :] for c in range(CT)]

            mview = mtre_d.rearrange("(m p) t -> p m t", p=P)

            # critical first loads on two queues in parallel
            nc.gpsimd.dma_start(out=zre_c[0], in_=zre_d[0:P, :])
            nc.sync.dma_start(out=mtre0, in_=mtre_d[0:P, :])
            nc.sync.dma_start(out=mtre123, in_=mview[:, 1:4, :])
            for c in range(1, CT):
                nc.sync.dma_start(out=zre_c[c],
                                  in_=zre_d[c * P:(c + 1) * P, :])
            nc.gpsimd.dma_start(out=ident, in_=ident_d)
            nc.gpsimd.dma_start(out=tri, in_=tri_d)
            nview = ntre_d.rearrange("(m p) t -> p m t", p=P)
            nc.sync.dma_start(out=ntre_sb, in_=nview)
            for c in range(CT):
                nc.sync.dma_start(out=zim_c[c],
                                  in_=zim_d[c * P:(c + 1) * P, :])
            if has_imag:
                def load4(dram, tag):
                    t4 = cp.tile([P, CT, C], bf16, tag=tag, name=tag)
                    nc.sync.dma_start(
                        out=t4, in_=dram.rearrange("(m p) t -> p m t", p=P))
                    return [t4[:, c, :] for c in range(CT)]
                mtim = load4(mtim_d, "mtim")
                mtimn = load4(mtimn_d, "mtimn")
                ntim = load4(ntim_d, "ntim")
                ntimn = load4(ntimn_d, "ntimn")

            # persistent result tiles (y split by column half: precise deps)
            yre = [[cp.tile([P, 512], bf16, tag=f"yre{c}_{n}",
                            name=f"yre{c}_{n}") for n in range(2)]
                   for c in range(CT)]
            yim = [[cp.tile([P, 512], bf16, tag=f"yim{c}_{n}",
                            name=f"yim{c}_{n}") for n in range(2)]
                   for c in range(CT)]
            ure = [cp.tile([P, C], bf16, tag=f"ure{j}", name=f"ure{j}")
                   for j in range(TT)]
            uim = [cp.tile([P, C], bf16, tag=f"uim{j}", name=f"uim{j}")
                   for j in range(TT)]

            def emit_y(dst, terms):
                """dst[m][n] = sum_terms w^T z ([C,T] channel-major)."""
                nterm = len(terms)
                for n in range(2):
                    pss = [pmm.tile([P, 512], f32, tag="mm", name="psmm")
                           for _ in range(CT)]
                    for t_i, (w, z) in enumerate(terms):
                        for c in range(CT):
                            for m in range(CT):
                                _mm(nc, pss[m], w[c][:, m * P:(m + 1) * P],
                                    z[c][:, n * 512:(n + 1) * 512],
                                    start=(t_i == 0 and c == 0),
                                    stop=(t_i == nterm - 1 and c == CT - 1))
                    for m in range(CT):
                        nc.vector.tensor_copy(out=dst[m][n], in_=pss[m])

            def emit_u(dst, terms):
                """dst[j] = sum_terms z^T w ([T,C] token-major)."""
                for j in range(TT):
                    jsl = slice(j * P, (j + 1) * P)
                    ps = pmm.tile([P, 512], f32, tag="mm", name="psmm")
                    nacc = len(terms) * CT
                    k = 0
                    for z, w in terms:
                        for c in range(CT):
                            _mm(nc, ps, z[c][:, jsl], w[c],
                                start=(k == 0), stop=(k == nacc - 1))
                            k += 1
                    nc.vector.tensor_copy(out=dst[j], in_=ps)

            # -- P^T blocks: one tile per t-chunk, j-blocks side by side ----
            pt0 = cp.tile([P, 4, 512], bf16, tag="pt0", name="pt0")
            pt1 = cp.tile([P, 8, 512], bf16, tag="pt1", name="pt1")
            ptn = (pt0, pt1)

            def emit_out_chunk(n, half, c0=0, c1=512):
                """out[:, n*512+c0 : n*512+c1] = U^T @ P^T for re/im."""
                tsl = slice(n * 512 + c0, n * 512 + c1)
                u, dram = ((ure, outre_d), (uim, outim_d))[half]
                dview = dram.rearrange("(m p) t -> p m t", p=P)
                js = [j for j in range(4 * n + 4) if j * P - n * 512 < c1]
                for mh in range(2):
                    o = wp.tile([P, 2, c1 - c0], bf16, tag=f"osb{c1 - c0}",
                                name="osb")
                    for mi in range(2):
                        m = 2 * mh + mi
                        msl = slice(m * P, (m + 1) * P)
                        ps = pmm.tile([P, 512], f32, tag="mm", name="psmm")
                        for j in js:
                            # pt j-block is zero left of column j*P-n*512
                            lo = max(c0, j * P - n * 512)
                            _mm(nc, ps[:, lo:c1],
                                u[j][:, msl], ptn[n][:, j, lo:c1],
                                start=(j == js[0]), stop=(j == js[-1]))
                        nc.vector.tensor_copy(out=o[:, mi, :],
                                              in_=ps[:, c0:c1])
                    nc.sync.dma_start(out=dview[:, 2 * mh:2 * mh + 2, tsl],
                                      in_=o)

            # -- scores / softmax / transposes per t-tile -------------------
            def emit_scores_tile(i):
                ui = (i + 1) * P
                isl = slice(i * P, (i + 1) * P)
                s_sb = wp.tile([P, T], bf16, tag="s", name="s_sb")
                nchunks = (ui + 511) // 512
                lparts = []
                for q in range(nchunks):
                    w = min(512, ui - q * 512)
                    ps = pmm.tile([P, 512], f32, tag="mm", name="psmm")
                    k = 0
                    for z, y in ((zre_c, yre), (zim_c, yim)):
                        for c in range(CT):
                            _mm(nc, ps[:, :w], z[c][:, isl], y[c][q][:, :w],
                                start=(k == 0), stop=(k == 2 * CT - 1))
                            k += 1
                    if q == nchunks - 1:
                        # causal frontier: add the tri mask in-place (PSUM)
                        nc.vector.tensor_add(out=ps[:, w - P: w],
                                             in0=ps[:, w - P: w], in1=tri)
                    lp = sp.tile([P, 1], f32, tag="lp", name="lp")
                    nc.scalar.activation(
                        out=s_sb[:, q * 512: q * 512 + w],
                        in_=ps[:, :w],
                        func=mybir.ActivationFunctionType.Exp,
                        accum_out=lp,
                    )
                    lparts.append(lp)

                lsum = lparts[0]
                for extra in lparts[1:]:
                    acc = sp.tile([P, 1], f32, tag="lacc", name="lacc")
                    nc.vector.tensor_add(out=acc, in0=lsum, in1=extra)
                    lsum = acc
                rl = sp.tile([P, 1], f32, tag="rl", name="rl")
                nc.vector.reciprocal(out=rl, in_=lsum)

                # fold 1/l into the transpose: block^T @ diag(1/l) on the PE
                dg = sp.tile([P, P], bf16, tag="dg", name="dg")
                nc.vector.tensor_scalar_mul(dg, ident, rl)
                n = i // 4
                lc = i * P - n * 512
                for g in range(0, i + 1, 4):
                    cnt = min(4, i + 1 - g)
                    psq = ptr.tile([P, 4, P], f32, tag="tr", name="pstile")
                    for k in range(cnt):
                        j = g + k
                        _mm(nc, psq[:, k, :], s_sb[:, j * P:(j + 1) * P], dg,
                            start=True, stop=True)
                    nc.vector.tensor_copy(
                        out=ptn[n][:, g:g + cnt, lc:lc + P],
                        in_=psq[:, :cnt, :],
                    )

            # -- phase order ------------------------------------------------
            if not has_imag:
                emit_y(yre, [(mt_aps, zre_c)])
                emit_u(ure, [(zre_c, nt_aps)])
                emit_y(yim, [(mt_aps, zim_c)])
                emit_u(uim, [(zim_c, nt_aps)])
            else:
                emit_y(yre, [(mt_aps, zre_c), (mtimn, zim_c)])
                emit_u(ure, [(zre_c, nt_aps), (zim_c, ntimn)])
                emit_y(yim, [(mt_aps, zim_c), (mtim, zre_c)])
                emit_u(uim, [(zim_c, nt_aps), (zre_c, ntim)])

            for i in (3, 2, 1, 0, 4, 5):
                emit_scores_tile(i)
            emit_out_chunk(0, half=0)
            emit_scores_tile(6)
            emit_out_chunk(0, half=1)
            emit_scores_tile(7)
            # out chunk 1 split by column strip: cols 0:384 need only
            # scores tiles 4..6, so they run while tile 7's softmax drains
            emit_out_chunk(1, half=0, c0=0, c1=384)
            emit_out_chunk(1, half=1, c0=0, c1=384)
            emit_out_chunk(1, half=0, c0=384, c1=512)
            emit_out_chunk(1, half=1, c0=384, c1=512)

    nc.compile()
    return nc


def _prep_weights(Wq, phi_q, Wk, phi_k, Wv, phi_v, Wo, phi_o):
    Wq, Wk, Wv, Wo = (np.asarray(w, np.float64) for w in (Wq, Wk, Wv, Wo))
    pq, pk, pv, po = (np.asarray(p, np.float64)
                      for p in (phi_q, phi_k, phi_v, phi_o))
    M = (Wq.T @ (np.exp(1j * (pk - pq))[:, None] * Wk)) / math.sqrt(DH)
    N = (np.exp(1j * po)[:, None] * Wo) @ (np.exp(1j * pv)[:, None] * Wv)
    has_imag = not (np.allclose(M.imag, 0.0) and np.allclose(N.imag, 0.0))
    return M, N, has_imag


def kernel(z_re, z_im, Wq, phi_q, Wk, phi_k, Wv, phi_v, Wo, phi_o):
    import ml_dtypes
    snp = ml_dtypes.bfloat16
    z_re = np.ascontiguousarray(np.asarray(z_re, np.float32).astype(snp))
    z_im = np.ascontiguousarray(np.asarray(z_im, np.float32).astype(snp))
    M, N, has_imag = _prep_weights(Wq, phi_q, Wk, phi_k, Wv, phi_v, Wo, phi_o)

    mtre = np.ascontiguousarray(M.real.T.astype(snp))
    ntre = np.ascontiguousarray(N.real.T.astype(snp))
    consts = {"mtre": mtre, "ntre": ntre}
    if has_imag:
        mtim = np.ascontiguousarray(M.imag.T.astype(snp))
        ntim = np.ascontiguousarray(N.imag.T.astype(snp))
        consts.update(mtim=mtim, mtimn=-mtim, ntim=ntim, ntimn=-ntim)

    consts["ident"] = np.eye(P, dtype=snp)
    consts["tri"] = np.triu(np.full((P, P), NEG, np.float32), 1)
    nc = _get_program(has_imag)
    in_maps = [
        dict(consts, zre=z_re[b].reshape(C, T), zim=z_im[b].reshape(C, T))
        for b in range(B)
    ]
    res = run_bass_kernel_spmd(nc, in_maps, list(range(B)))
    out_re = np.stack([res.results[b]["outre"].astype(np.float32)
                       .reshape(C, HH, WW) for b in range(B)])
    out_im = np.stack([res.results[b]["outim"].astype(np.float32)
                       .reshape(C, HH, WW) for b in range(B)])
    return out_re, out_im


# revision 12
# speedup vs baseline: 1.0254x; 1.0062x over previous
"""Trainium2 Bass kernel for nn_ComplexAttention (B=8, C=512, H=W=32, HEADS=8).

Strategy
--------
Data-parallel over batch: one batch element per NeuronCore (8 cores), no
collectives.  Host-side algebraic fusion shrinks the per-core work:

  reference:  Q = R_q Wq Z,  K = R_k Wk Z,  V = R_v Wv Z   (complex, [C,T])
              S = Re(Q^H K)/sqrt(dh),  causal softmax -> A
              out = R_o Wo (V A^T)

  fused:      M = Wq^T diag(e^{i(phi_k-phi_q)}) Wk / sqrt(dh)   (host, f64)
              N = diag(e^{i phi_o}) Wo diag(e^{i phi_v}) Wv     (host, f64)
              Y = M Z            (channel-major [C,T])
              S = Re(Z^H Y)      = Zre^T Yre + Zim^T Yim
              A = softmax(causal(S))        (no max-subtraction: |S| < ~30)
              U = N Z            (token-major [T,C])
              out = U^T A^T      (channel-major [C,T], = re/im pair)

Whole datapath is bf16 (PSUM accumulation f32): 1 cyc/row PE rate, half
the DMA/DVE bytes, exact causal clamping (no N>=256 f32r restriction).
Outputs are written bf16 and upcast on host (rel err ~8e-3 vs 2e-2 gate).

Schedule notes (from HW traces):
 - NEFF bootstrap is ~7us; DMA_DIRECT2D issue is ~0.6us each, so inputs
   are consolidated into 13 large DMAs (zre/zim as [128,1024] c-tiles,
   mtre as 1+3 row-tiles, ntre as one [128,4,512]).  The two tiles the
   first matmul needs (mtre0 / zre c0) go first on separate queues.
 - phase order: Yre, Ure, Yim, Uim (ntre is loaded before zim), then
   scores tiles 7,6,5,4 / 3 / out(1,h0) / 2 / out(1,h1) / 1,0 / out(0);
   the interleaved scores tiles hide softmax latency ahead of each out
   chunk, and out(1) drains its DMA under the scores 0..3 work.
 - softmax exp reads scores straight out of PSUM (no copy), per-chunk
   partial row-sums are added on DVE afterwards.
"""

import math

import numpy as np

import concourse.mybir as mybir
import concourse.tile as tile
from concourse import bacc
from concourse.bass_utils import run_bass_kernel_spmd

B, C, HH, WW = 8, 512, 32, 32
T = HH * WW          # 1024 tokens
DH = C // 8          # head dim (scale only)
P = 128
CT = C // P          # 4 channel tiles
TT = T // P          # 8 token tiles
NEG = -1.0e30

f32 = mybir.dt.float32
bf16 = mybir.dt.bfloat16


def _mm(nc, out, lhsT, rhs, start, stop):
    nc.tensor.matmul(out, lhsT, rhs, start=start, stop=stop)


_CACHE: dict = {}


def _get_program(has_imag: bool):
    key = has_imag
    if key not in _CACHE:
        _CACHE[key] = _build_program(has_imag)
    return _CACHE[key]


def _build_program(has_imag: bool):
    nc = bacc.Bacc("TRN2", target_bir_lowering=False, debug=False)

    zre_d = nc.dram_tensor("zre", [C, T], bf16, kind="ExternalInput").ap()
    zim_d = nc.dram_tensor("zim", [C, T], bf16, kind="ExternalInput").ap()
    mtre_d = nc.dram_tensor("mtre", [C, C], bf16, kind="ExternalInput").ap()
    ntre_d = nc.dram_tensor("ntre", [C, C], bf16, kind="ExternalInput").ap()
    if has_imag:
        mtim_d = nc.dram_tensor("mtim", [C, C], bf16, kind="ExternalInput").ap()
        mtimn_d = nc.dram_tensor("mtimn", [C, C], bf16, kind="ExternalInput").ap()
        ntim_d = nc.dram_tensor("ntim", [C, C], bf16, kind="ExternalInput").ap()
        ntimn_d = nc.dram_tensor("ntimn", [C, C], bf16, kind="ExternalInput").ap()
    ident_d = nc.dram_tensor("ident", [P, P], bf16, kind="ExternalInput").ap()
    tri_d = nc.dram_tensor("tri", [P, P], f32, kind="ExternalInput").ap()
    outre_d = nc.dram_tensor("outre", [C, T], bf16, kind="ExternalOutput").ap()
    outim_d = nc.dram_tensor("outim", [C, T], bf16, kind="ExternalOutput").ap()

    with tile.TileContext(nc) as tc:
        with (
            tc.tile_pool(name="const", bufs=1) as cp,
            tc.tile_pool(name="work", bufs=4) as wp,
            tc.tile_pool(name="small", bufs=12) as sp,
            tc.tile_pool(name="psmm", bufs=6, space="PSUM") as pmm,
            tc.tile_pool(name="pstr", bufs=2, space="PSUM") as ptr,
        ):
            # -- input tiles -----------------------------------------------
            zre_c = [cp.tile([P, T], bf16, tag=f"zre{c}", name=f"zre{c}")
                     for c in range(CT)]
            zim_c = [cp.tile([P, T], bf16, tag=f"zim{c}", name=f"zim{c}")
                     for c in range(CT)]
            mtre0 = cp.tile([P, C], bf16, tag="mtre0", name="mtre0")
            mtre123 = cp.tile([P, 3, C], bf16, tag="mtre123", name="mtre123")
            ntre_sb = cp.tile([P, CT, C], bf16, tag="ntre", name="ntre")
            ident = cp.tile([P, P], bf16, tag="ident", name="ident")
            tri = cp.tile([P, P], f32, tag="tri", name="tri")

            mt_aps = [mtre0] + [mtre123[:, i, :] for i in range(3)]
            nt_aps = [ntre_sb[:, c, :] for c in range(CT)]

            mview = mtre_d.rearrange("(m p) t -> p m t", p=P)

            # critical first loads: the first Y matmuls need only
            # zre c0 half 0 + mtre0 — put those first on the sync queue
            # (it issues earliest); the other zre c0 half goes on gpsimd.
            nc.sync.dma_start(out=zre_c[0][:, 0:512], in_=zre_d[0:P, 0:512])
            nc.sync.dma_start(out=mtre0, in_=mtre_d[0:P, :])
            nc.gpsimd.dma_start(out=zre_c[0][:, 512:T],
                                in_=zre_d[0:P, 512:T])
            nc.sync.dma_start(out=mtre123, in_=mview[:, 1:4, :])
            for c in range(1, CT):
                nc.sync.dma_start(out=zre_c[c],
                                  in_=zre_d[c * P:(c + 1) * P, :])
            nc.gpsimd.dma_start(out=ident, in_=ident_d)
            nc.gpsimd.dma_start(out=tri, in_=tri_d)
            nview = ntre_d.rearrange("(m p) t -> p m t", p=P)
            nc.sync.dma_start(out=ntre_sb, in_=nview)
            for c in range(CT):
                nc.sync.dma_start(out=zim_c[c],
                                  in_=zim_d[c * P:(c + 1) * P, :])
            if has_imag:
                def load4(dram, tag):
                    t4 = cp.tile([P, CT, C], bf16, tag=tag, name=tag)
                    nc.sync.dma_start(
                        out=t4, in_=dram.rearrange("(m p) t -> p m t", p=P))
                    return [t4[:, c, :] for c in range(CT)]
                mtim = load4(mtim_d, "mtim")
                mtimn = load4(mtimn_d, "mtimn")
                ntim = load4(ntim_d, "ntim")
                ntimn = load4(ntimn_d, "ntimn")

            # persistent result tiles (y split by column half: precise deps)
            yre = [[cp.tile([P, 512], bf16, tag=f"yre{c}_{n}",
                            name=f"yre{c}_{n}") for n in range(2)]
                   for c in range(CT)]
            yim = [[cp.tile([P, 512], bf16, tag=f"yim{c}_{n}",
                            name=f"yim{c}_{n}") for n in range(2)]
                   for c in range(CT)]
            ure = [cp.tile([P, C], bf16, tag=f"ure{j}", name=f"ure{j}")
                   for j in range(TT)]
            uim = [cp.tile([P, C], bf16, tag=f"uim{j}", name=f"uim{j}")
                   for j in range(TT)]

            def emit_y(dst, terms):
                """dst[m][n] = sum_terms w^T z ([C,T] channel-major)."""
                nterm = len(terms)
                for n in range(2):
                    pss = [pmm.tile([P, 512], f32, tag="mm", name="psmm")
                           for _ in range(CT)]
                    for t_i, (w, z) in enumerate(terms):
                        for c in range(CT):
                            for m in range(CT):
                                _mm(nc, pss[m], w[c][:, m * P:(m + 1) * P],
                                    z[c][:, n * 512:(n + 1) * 512],
                                    start=(t_i == 0 and c == 0),
                                    stop=(t_i == nterm - 1 and c == CT - 1))
                    for m in range(CT):
                        nc.vector.tensor_copy(out=dst[m][n], in_=pss[m])

            def emit_u(dst, terms):
                """dst[j] = sum_terms z^T w ([T,C] token-major)."""
                for j in range(TT):
                    jsl = slice(j * P, (j + 1) * P)
                    ps = pmm.tile([P, 512], f32, tag="mm", name="psmm")
                    nacc = len(terms) * CT
                    k = 0
                    for z, w in terms:
                        for c in range(CT):
                            _mm(nc, ps, z[c][:, jsl], w[c],
                                start=(k == 0), stop=(k == nacc - 1))
                            k += 1
                    nc.vector.tensor_copy(out=dst[j], in_=ps)

            # -- P^T blocks: one tile per t-chunk, j-blocks side by side ----
            pt0 = cp.tile([P, 4, 512], bf16, tag="pt0", name="pt0")
            pt1 = cp.tile([P, 8, 512], bf16, tag="pt1", name="pt1")
            ptn = (pt0, pt1)

            def emit_out_chunk(n, half, c0=0, c1=512):
                """out[:, n*512+c0 : n*512+c1] = U^T @ P^T for re/im."""
                tsl = slice(n * 512 + c0, n * 512 + c1)
                u, dram = ((ure, outre_d), (uim, outim_d))[half]
                dview = dram.rearrange("(m p) t -> p m t", p=P)
                js = [j for j in range(4 * n + 4) if j * P - n * 512 < c1]
                for mh in range(2):
                    o = wp.tile([P, 2, c1 - c0], bf16, tag=f"osb{c1 - c0}",
                                name="osb")
                    for mi in range(2):
                        m = 2 * mh + mi
                        msl = slice(m * P, (m + 1) * P)
                        ps = pmm.tile([P, 512], f32, tag="mm", name="psmm")
                        for j in js:
                            # pt j-block is zero left of column j*P-n*512
                            lo = max(c0, j * P - n * 512)
                            _mm(nc, ps[:, lo:c1],
                                u[j][:, msl], ptn[n][:, j, lo:c1],
                                start=(j == js[0]), stop=(j == js[-1]))
                        nc.vector.tensor_copy(out=o[:, mi, :],
                                              in_=ps[:, c0:c1])
                    nc.sync.dma_start(out=dview[:, 2 * mh:2 * mh + 2, tsl],
                                      in_=o)

            # -- scores / softmax / transposes per t-tile -------------------
            def emit_scores_tile(i):
                ui = (i + 1) * P
                isl = slice(i * P, (i + 1) * P)
                s_sb = wp.tile([P, T], bf16, tag="s", name="s_sb")
                nchunks = (ui + 511) // 512
                lparts = []
                for q in range(nchunks):
                    w = min(512, ui - q * 512)
                    ps = pmm.tile([P, 512], f32, tag="mm", name="psmm")
                    k = 0
                    for z, y in ((zre_c, yre), (zim_c, yim)):
                        for c in range(CT):
                            _mm(nc, ps[:, :w], z[c][:, isl], y[c][q][:, :w],
                                start=(k == 0), stop=(k == 2 * CT - 1))
                            k += 1
                    if q == nchunks - 1:
                        # causal frontier: add the tri mask in-place (PSUM)
                        nc.vector.tensor_add(out=ps[:, w - P: w],
                                             in0=ps[:, w - P: w], in1=tri)
                    lp = sp.tile([P, 1], f32, tag="lp", name="lp")
                    nc.scalar.activation(
                        out=s_sb[:, q * 512: q * 512 + w],
                        in_=ps[:, :w],
                        func=mybir.ActivationFunctionType.Exp,
                        accum_out=lp,
                    )
                    lparts.append(lp)

                lsum = lparts[0]
                for extra in lparts[1:]:
                    acc = sp.tile([P, 1], f32, tag="lacc", name="lacc")
                    nc.vector.tensor_add(out=acc, in0=lsum, in1=extra)
                    lsum = acc
                rl = sp.tile([P, 1], f32, tag="rl", name="rl")
                nc.vector.reciprocal(out=rl, in_=lsum)

                # fold 1/l into the transpose: block^T @ diag(1/l) on the PE
                dg = sp.tile([P, P], bf16, tag="dg", name="dg")
                nc.vector.tensor_scalar_mul(dg, ident, rl)
                n = i // 4
                lc = i * P - n * 512
                for g in range(0, i + 1, 4):
                    cnt = min(4, i + 1 - g)
                    psq = ptr.tile([P, 4, P], f32, tag="tr", name="pstile")
                    for k in range(cnt):
                        j = g + k
                        _mm(nc, psq[:, k, :], s_sb[:, j * P:(j + 1) * P], dg,
                            start=True, stop=True)
                    nc.vector.tensor_copy(
                        out=ptn[n][:, g:g + cnt, lc:lc + P],
                        in_=psq[:, :cnt, :],
                    )

            # -- phase order ------------------------------------------------
            if not has_imag:
                emit_y(yre, [(mt_aps, zre_c)])
                emit_u(ure, [(zre_c, nt_aps)])
                emit_y(yim, [(mt_aps, zim_c)])
                emit_u(uim, [(zim_c, nt_aps)])
            else:
                emit_y(yre, [(mt_aps, zre_c), (mtimn, zim_c)])
                emit_u(ure, [(zre_c, nt_aps), (zim_c, ntimn)])
                emit_y(yim, [(mt_aps, zim_c), (mtim, zre_c)])
                emit_u(uim, [(zim_c, nt_aps), (zre_c, ntim)])

            for i in (3, 2, 1, 0, 4, 5):
                emit_scores_tile(i)
            emit_out_chunk(0, half=0)
            emit_scores_tile(6)
            emit_out_chunk(0, half=1)
            emit_scores_tile(7)
            # out chunk 1 split by column strip: cols 0:384 need only
            # scores tiles 4..6, so they run while tile 7's softmax drains
            emit_out_chunk(1, half=0, c0=0, c1=384)
            emit_out_chunk(1, half=1, c0=0, c1=384)
            emit_out_chunk(1, half=0, c0=384, c1=512)
            emit_out_chunk(1, half=1, c0=384, c1=512)

    nc.compile()
    return nc


def _prep_weights(Wq, phi_q, Wk, phi_k, Wv, phi_v, Wo, phi_o):
    Wq, Wk, Wv, Wo = (np.asarray(w, np.float64) for w in (Wq, Wk, Wv, Wo))
    pq, pk, pv, po = (np.asarray(p, np.float64)
                      for p in (phi_q, phi_k, phi_v, phi_o))
    M = (Wq.T @ (np.exp(1j * (pk - pq))[:, None] * Wk)) / math.sqrt(DH)
    N = (np.exp(1j * po)[:, None] * Wo) @ (np.exp(1j * pv)[:, None] * Wv)
    has_imag = not (np.allclose(M.imag, 0.0) and np.allclose(N.imag, 0.0))
    return M, N, has_imag


def kernel(z_re, z_im, Wq, phi_q, Wk, phi_k, Wv, phi_v, Wo, phi_o):
    import ml_dtypes
    snp = ml_dtypes.bfloat16
    z_re = np.ascontiguousarray(np.asarray(z_re, np.float32).astype(snp))
    z_im = np.ascontiguousarray(np.asarray(z_im, np.float32).astype(snp))
    M, N, has_imag = _prep_weights(Wq, phi_q, Wk, phi_k, Wv, phi_v, Wo, phi_o)

    mtre = np.ascontiguousarray(M.real.T.astype(snp))
    ntre = np.ascontiguousarray(N.real.T.astype(snp))
    consts = {"mtre": mtre, "ntre": ntre}
    if has_imag:
        mtim = np.ascontiguousarray(M.imag.T.astype(snp))
        ntim = np.ascontiguousarray(N.imag.T.astype(snp))
        consts.update(mtim=mtim, mtimn=-mtim, ntim=ntim, ntimn=-ntim)

    consts["ident"] = np.eye(P, dtype=snp)
    consts["tri"] = np.triu(np.full((P, P), NEG, np.float32), 1)
    nc = _get_program(has_imag)
    in_maps = [
        dict(consts, zre=z_re[b].reshape(C, T), zim=z_im[b].reshape(C, T))
        for b in range(B)
    ]
    res = run_bass_kernel_spmd(nc, in_maps, list(range(B)))
    out_re = np.stack([res.results[b]["outre"].astype(np.float32)
                       .reshape(C, HH, WW) for b in range(B)])
    out_im = np.stack([res.results[b]["outim"].astype(np.float32)
                       .reshape(C, HH, WW) for b in range(B)])
    return out_re, out_im
